# revision 2
# baseline (speedup 1.0000x reference)
"""Trainium2 Bass kernel for the GNN message-passing layer (nn_GNN_layer_60610578482039).

Math (per graph g, n=512 nodes, C=32 in-feats, B=64 out-feats):
    ret = A_t @ X1^T / n + X2^T, with A_t = c0*A + const + vec_i + vec_j and
    X1/X2 linear in the basis [X^T, mean_X, mean_cols, diag, mean_diag, mean_all].

Because A_t and X1/X2 are affine in A-contractions, the whole layer folds into
    ret^T[b,i] = sum_j RH1[j,b] * A^T[j,i]  +  sum_k L[k,b] * G2[k,i]
where RH1 = [X | mean_cols | diag | 1] @ H1 (n x B) is a cheap host-side fold,
and the second (A-independent, rank-34) term has L = [H35; H67; H68] (34 x B)
and G2 = [X^T; diag; 1] (34 x n).  Both terms are PE accumulations into one
PSUM bank: 4 j-tiles of 128 over A^T plus one K=34 tile — no DVE add and no
f32 `base` DMA.

Sharding: data-parallel over the batch dim N=64 -> 8 graphs per NeuronCore.
Per graph: one DMA of [128, 4, 576] bf16 (cols 0:512 = A^T j-tile rows, cols
512:576 = RH1 rows), 5 accumulating PE matmuls, a DVE copy PSUM -> SBUF bf16,
and one shared out-DMA per 2 graphs.  In-DMAs issue from SP, out-DMAs from the
Activation engine so descriptor generation overlaps.  Output travels as bf16
out^T [64, NG, 512]; the host transposes/casts to [N, 512, 64] f32 at gather.
"""

import numpy as np
import ml_dtypes

N, NNODES, CIN, COUT = 64, 512, 32, 64
NCORES = 8
NG = N // NCORES  # graphs per core
JT = NNODES // 128  # j-tiles per graph
KX = CIN + 2  # rank of the A-independent term: [X^T; diag; 1]

# test.py can flip these before calling kernel()
TRACE = False
LAST_RESULTS = None  # BassKernelResults of the last run

_NC_CACHE = {}


def _host_fold(A, X, c, W1, W2):
    """Fold all parameter-side algebra on host (f32 — device bf16 dominates error).

    Returns (atr [N,128,JT,576] bf16, ext [KX, N, 576] bf16).

    G^T row order for the factored product ret^T = H^T @ G (K=69):
      rows 0:32  (A@X)^T      -> H[c]  = (c0/n) W1x^T
      row  32    (A@mc)^T     -> H     = (c0/n) w1mc
      row  33    (A@diag)^T   -> H     = (c0/n) w1d
      row  34    rowsum^T     -> H     = (c0/n) a1 + (w2mc + c3*S1/n)/n
      rows 35:67 X^T          -> H     = W2x^T + outer(w6, S1/n)
      row  67    diag         -> H     = w2d + c4*S1/n
      row  68    ones         -> H     = const*S1/n + S2/n + a2
    Rows 0:35 equal R^T @ A^T with R = [X | mc | diag | 1], so they fold into
    RH1 = R @ H[0:35] and ride in atr cols 512:576; rows 35:69 are
    A-independent and ride in ext as a K=34 accumulation tile.
    """
    n, C = NNODES, CIN
    f = np.float32
    bf16 = ml_dtypes.bfloat16
    c = c.astype(f)
    w6 = c[5 : 5 + C]
    w7 = c[5 + C : 5 + 2 * C]
    c0, c1, c2, c3, c4 = c[0], c[1], c[2], c[3], c[4]
    W1 = W1.astype(f)
    W2 = W2.astype(f)
    w1x, w1m = W1[:, :C], W1[:, C : 2 * C]
    w1mc, w1d, w1md, w1ma = W1[:, 2 * C], W1[:, 2 * C + 1], W1[:, 2 * C + 2], W1[:, 2 * C + 3]
    w2x, w2m = W2[:, :C], W2[:, C : 2 * C]
    w2mc, w2d, w2md, w2ma = W2[:, 2 * C], W2[:, 2 * C + 1], W2[:, 2 * C + 2], W2[:, 2 * C + 3]

    Af = np.ascontiguousarray(A, dtype=f)
    Xf = np.ascontiguousarray(X, dtype=f)
    rowsums = Af.sum(axis=2)  # [N, n]
    mc = rowsums / n
    diag = np.einsum("gii->gi", Af).copy()  # [N, n]
    mean_diag = diag.mean(axis=1)  # [N]
    mean_all = rowsums.sum(axis=1) / (n * n)  # [N]
    mean_X = Xf.mean(axis=1)  # [N, C]

    a1 = mean_X @ w1m.T + np.outer(mean_diag, w1md) + np.outer(mean_all, w1ma)  # [N, B]
    a2 = mean_X @ w2m.T + np.outer(mean_diag, w2md) + np.outer(mean_all, w2ma)
    S1 = n * (mean_X @ w1x.T) + n * np.outer(mean_all, w1mc) + n * np.outer(mean_diag, w1d) + n * a1
    s = Xf @ w6  # [N, n]
    vec = c3 * mc + c4 * diag + s  # [N, n]
    vX = np.einsum("gn,gnc->gc", vec, Xf)  # [N, C]
    S2 = (
        vX @ w1x.T
        + np.outer(np.einsum("gn,gn->g", vec, mc), w1mc)
        + np.outer(np.einsum("gn,gn->g", vec, diag), w1d)
        + vec.sum(axis=1)[:, None] * a1
    )
    const = c1 * mean_all + c2 * mean_diag + mean_X @ w7  # [N]

    # H1 rows (0:35) -> RH1 fold via one batched matmul:
    # RH1 = [X | mc | diag | 1] @ [H0; H32; H33; H34]
    H34 = (c0 / n) * a1 + (w2mc[None, :] + c3 * S1 / n) / n  # [N, B]
    Raug = np.empty((N, n, C + 3), dtype=f)
    Raug[:, :, :C] = Xf
    Raug[:, :, C] = mc
    Raug[:, :, C + 1] = diag
    Raug[:, :, C + 2] = 1.0
    H1aug = np.empty((N, C + 3, COUT), dtype=f)
    H1aug[:, :C, :] = (c0 / n) * w1x.T[None]
    H1aug[:, C, :] = (c0 / n) * w1mc[None]
    H1aug[:, C + 1, :] = (c0 / n) * w1d[None]
    H1aug[:, C + 2, :] = H34
    RH1 = Raug @ H1aug  # [N, n, B]

    # H2 rows (35:69) -> ext fold: lhsT = [H35; H67; H68], rhs = [X^T; diag; 1]
    H35 = w2x.T[None] + w6[None, :, None] * (S1[:, None, :] / n)  # [N, C, B]
    H67 = w2d[None, :] + c4 * S1 / n  # [N, B]
    H68 = const[:, None] * S1 / n + S2 / n + a2  # [N, B]

    # Pack [A^T | RH1] -> [N, 128, JT, 576] bf16 (atr[g,p,jt,i] = A[g,i,jt*128+p])
    atr = np.empty((N, 128, JT, 576), dtype=bf16)
    for jt in range(JT):
        atr[:, :, jt, :NNODES] = Af[:, :, jt * 128 : (jt + 1) * 128].swapaxes(1, 2)
        atr[:, :, jt, NNODES:] = RH1[:, jt * 128 : (jt + 1) * 128, :]

    # Pack ext [KX, N, 576]: cols 0:512 = rhs rows, cols 512:576 = lhsT rows
    ext = np.empty((KX, N, 576), dtype=bf16)
    ext[:C, :, :NNODES] = Xf.transpose(2, 0, 1)
    ext[C, :, :NNODES] = diag
    ext[C + 1, :, :NNODES] = 1.0
    ext[:C, :, NNODES:] = H35.transpose(1, 0, 2)
    ext[C, :, NNODES:] = H67
    ext[C + 1, :, NNODES:] = H68
    return atr, ext


def _build_nc():
    import concourse.tile as tile
    from concourse import bacc, mybir

    nc = bacc.Bacc("TRN2", target_bir_lowering=False, debug=False)
    atr = nc.dram_tensor(
        "atr", [NG, 128, JT, 576], mybir.dt.bfloat16, kind="ExternalInput"
    ).ap()
    ext = nc.dram_tensor(
        "ext", [KX, NG, 576], mybir.dt.bfloat16, kind="ExternalInput"
    ).ap()
    outb = nc.dram_tensor(
        "outb", [COUT, NG, NNODES], mybir.dt.bfloat16, kind="ExternalOutput"
    ).ap()

    with tile.TileContext(nc) as tc:
        with (
            tc.tile_pool(name="io", bufs=3) as iop,
            tc.tile_pool(name="ex", bufs=1) as exp_,
            tc.tile_pool(name="ps", bufs=4, space="PSUM") as psp,
            tc.tile_pool(name="ob", bufs=2) as obp,
        ):
            et = exp_.tile([KX, NG, 576], mybir.dt.bfloat16, tag="ext")
            ot = None
            for g in range(NG):
                t = iop.tile([128, JT, 576], mybir.dt.bfloat16, tag="atr")
                nc.sync.dma_start(out=t[:], in_=atr[g])
                if g == 0:
                    # after the first atr DMA so graph 0's matmuls start ASAP
                    nc.sync.dma_start(out=et[:], in_=ext[:])
                ps = psp.tile([COUT, NNODES], mybir.dt.float32, tag="ps")
                for jt in range(JT):
                    nc.tensor.matmul(
                        ps[:],
                        lhsT=t[:, jt, NNODES:],
                        rhs=t[:, jt, 0:NNODES],
                        start=(jt == 0),
                        stop=False,
                    )
                nc.tensor.matmul(
                    ps[:],
                    lhsT=et[:, g, NNODES:],
                    rhs=et[:, g, 0:NNODES],
                    start=False,
                    stop=True,
                )
                if g % 2 == 0:
                    ot = obp.tile([COUT, 2, NNODES], mybir.dt.bfloat16, tag="out")
                nc.vector.tensor_copy(ot[:, g % 2, :], ps[:])
                if g % 2 == 1:
                    nc.scalar.dma_start(out=outb[:, g - 1 : g + 1, :], in_=ot[:])
    nc.compile()
    return nc


def kernel(A, X, A_coeffs, X_coeffs_1, X_coeffs_2):
    global LAST_RESULTS
    from concourse.bass_utils import run_bass_kernel_spmd

    atr, ext = _host_fold(
        np.asarray(A), np.asarray(X), np.asarray(A_coeffs),
        np.asarray(X_coeffs_1), np.asarray(X_coeffs_2),
    )

    if "nc" not in _NC_CACHE:
        _NC_CACHE["nc"] = _build_nc()
    nc = _NC_CACHE["nc"]

    in_maps = [
        {
            "atr": atr[c * NG : (c + 1) * NG],
            "ext": np.ascontiguousarray(ext[:, c * NG : (c + 1) * NG, :]),
        }
        for c in range(NCORES)
    ]
    res = run_bass_kernel_spmd(nc, in_maps, list(range(NCORES)), trace=TRACE)
    LAST_RESULTS = res
    outT = np.stack([r["outb"] for r in res.results])  # [ncores, B, NG, n]
    out = outT.transpose(0, 2, 3, 1).reshape(N, NNODES, COUT)  # [N, n, B]
    return np.ascontiguousarray(out).astype(np.float32)


# revision 8
# speedup vs baseline: 1.1640x; 1.1640x over previous
"""Trainium2 Bass kernel for the GNN message-passing layer (nn_GNN_layer_60610578482039).

Math (per graph g, n=512 nodes, C=32 in-feats, B=64 out-feats):
    ret = A_t @ X1^T / n + X2^T, with A_t = c0*A + const + vec_i + vec_j and
    X1/X2 linear in the basis [X^T, mean_X, mean_cols, diag, mean_diag, mean_all].

Because A_t and X1/X2 are affine in A-contractions, the whole layer folds into
    ret^T[b,i] = sum_j RH1[j,b] * A^T[j,i]  +  sum_k L[k,b] * G2[k,i]
where RH1 = [X | mean_cols | diag | 1] @ H1 (n x B) is a cheap host-side fold,
and the second (A-independent, rank-34) term has L = [H35; H67; H68] (34 x B)
and G2 = [X^T; diag; 1] (34 x n).  Both terms are PE accumulations into one
PSUM bank: 4 j-tiles of 128 over A^T plus one K=34 tile — no DVE add and no
f32 `base` DMA.

Sharding: data-parallel over the batch dim N=64 -> 8 graphs per NeuronCore.
Per graph: one DMA of [128, 4, 576] bf16 (cols 0:512 = A^T j-tile rows, cols
512:576 = RH1 rows), 5 accumulating PE matmuls, a DVE copy PSUM -> SBUF bf16,
and one shared out-DMA per 2 graphs.  In-DMAs issue from SP, out-DMAs from the
Activation engine so descriptor generation overlaps.  Output travels as bf16
out^T [64, NG, 512]; the host transposes/casts to [N, 512, 64] f32 at gather.
"""

import numpy as np
import ml_dtypes

N, NNODES, CIN, COUT = 64, 512, 32, 64
NCORES = 8
NG = N // NCORES  # graphs per core
JT = NNODES // 128  # j-tiles per graph
KX = CIN + 2  # rank of the A-independent term: [X^T; diag; 1]

# test.py can flip these before calling kernel()
TRACE = False
LAST_RESULTS = None  # BassKernelResults of the last run

_NC_CACHE = {}


def _host_fold(A, X, c, W1, W2):
    """Fold all parameter-side algebra on host (f32 — device bf16 dominates error).

    Returns (atr [N,128,JT,576] bf16, ext [KX, N, 576] bf16).

    G^T row order for the factored product ret^T = H^T @ G (K=69):
      rows 0:32  (A@X)^T      -> H[c]  = (c0/n) W1x^T
      row  32    (A@mc)^T     -> H     = (c0/n) w1mc
      row  33    (A@diag)^T   -> H     = (c0/n) w1d
      row  34    rowsum^T     -> H     = (c0/n) a1 + (w2mc + c3*S1/n)/n
      rows 35:67 X^T          -> H     = W2x^T + outer(w6, S1/n)
      row  67    diag         -> H     = w2d + c4*S1/n
      row  68    ones         -> H     = const*S1/n + S2/n + a2
    Rows 0:35 equal R^T @ A^T with R = [X | mc | diag | 1], so they fold into
    RH1 = R @ H[0:35] and ride in atr cols 512:576; rows 35:69 are
    A-independent and ride in ext as a K=34 accumulation tile.
    """
    n, C = NNODES, CIN
    f = np.float32
    bf16 = ml_dtypes.bfloat16
    c = c.astype(f)
    w6 = c[5 : 5 + C]
    w7 = c[5 + C : 5 + 2 * C]
    c0, c1, c2, c3, c4 = c[0], c[1], c[2], c[3], c[4]
    W1 = W1.astype(f)
    W2 = W2.astype(f)
    w1x, w1m = W1[:, :C], W1[:, C : 2 * C]
    w1mc, w1d, w1md, w1ma = W1[:, 2 * C], W1[:, 2 * C + 1], W1[:, 2 * C + 2], W1[:, 2 * C + 3]
    w2x, w2m = W2[:, :C], W2[:, C : 2 * C]
    w2mc, w2d, w2md, w2ma = W2[:, 2 * C], W2[:, 2 * C + 1], W2[:, 2 * C + 2], W2[:, 2 * C + 3]

    Af = np.ascontiguousarray(A, dtype=f)
    Xf = np.ascontiguousarray(X, dtype=f)
    rowsums = Af.sum(axis=2)  # [N, n]
    mc = rowsums / n
    diag = np.einsum("gii->gi", Af).copy()  # [N, n]
    mean_diag = diag.mean(axis=1)  # [N]
    mean_all = rowsums.sum(axis=1) / (n * n)  # [N]
    mean_X = Xf.mean(axis=1)  # [N, C]

    a1 = mean_X @ w1m.T + np.outer(mean_diag, w1md) + np.outer(mean_all, w1ma)  # [N, B]
    a2 = mean_X @ w2m.T + np.outer(mean_diag, w2md) + np.outer(mean_all, w2ma)
    S1 = n * (mean_X @ w1x.T) + n * np.outer(mean_all, w1mc) + n * np.outer(mean_diag, w1d) + n * a1
    s = Xf @ w6  # [N, n]
    vec = c3 * mc + c4 * diag + s  # [N, n]
    vX = np.einsum("gn,gnc->gc", vec, Xf)  # [N, C]
    S2 = (
        vX @ w1x.T
        + np.outer(np.einsum("gn,gn->g", vec, mc), w1mc)
        + np.outer(np.einsum("gn,gn->g", vec, diag), w1d)
        + vec.sum(axis=1)[:, None] * a1
    )
    const = c1 * mean_all + c2 * mean_diag + mean_X @ w7  # [N]

    # H1 rows (0:35) -> RH1 fold via one batched matmul:
    # RH1 = [X | mc | diag | 1] @ [H0; H32; H33; H34]
    H34 = (c0 / n) * a1 + (w2mc[None, :] + c3 * S1 / n) / n  # [N, B]
    Raug = np.empty((N, n, C + 3), dtype=f)
    Raug[:, :, :C] = Xf
    Raug[:, :, C] = mc
    Raug[:, :, C + 1] = diag
    Raug[:, :, C + 2] = 1.0
    H1aug = np.empty((N, C + 3, COUT), dtype=f)
    H1aug[:, :C, :] = (c0 / n) * w1x.T[None]
    H1aug[:, C, :] = (c0 / n) * w1mc[None]
    H1aug[:, C + 1, :] = (c0 / n) * w1d[None]
    H1aug[:, C + 2, :] = H34
    RH1 = Raug @ H1aug  # [N, n, B]

    # H2 rows (35:69) -> ext fold: lhsT = [H35; H67; H68], rhs = [X^T; diag; 1]
    H35 = w2x.T[None] + w6[None, :, None] * (S1[:, None, :] / n)  # [N, C, B]
    H67 = w2d[None, :] + c4 * S1 / n  # [N, B]
    H68 = const[:, None] * S1 / n + S2 / n + a2  # [N, B]

    # Pack [A^T | RH1] -> [N, 128, JT, 576] bf16 (atr[g,p,jt,i] = A[g,i,jt*128+p])
    atr = np.empty((N, 128, JT, 576), dtype=bf16)
    for jt in range(JT):
        atr[:, :, jt, :NNODES] = Af[:, :, jt * 128 : (jt + 1) * 128].swapaxes(1, 2)
        atr[:, :, jt, NNODES:] = RH1[:, jt * 128 : (jt + 1) * 128, :]

    # Pack ext [KX, N, 576]: cols 0:512 = rhs rows, cols 512:576 = lhsT rows
    ext = np.empty((KX, N, 576), dtype=bf16)
    ext[:C, :, :NNODES] = Xf.transpose(2, 0, 1)
    ext[C, :, :NNODES] = diag
    ext[C + 1, :, :NNODES] = 1.0
    ext[:C, :, NNODES:] = H35.transpose(1, 0, 2)
    ext[C, :, NNODES:] = H67
    ext[C + 1, :, NNODES:] = H68
    return atr, ext


def _build_nc():
    import concourse.tile as tile
    from concourse import bacc, mybir

    nc = bacc.Bacc("TRN2", target_bir_lowering=False, debug=False)
    atr = nc.dram_tensor(
        "atr", [NG, 128, JT, 576], mybir.dt.bfloat16, kind="ExternalInput"
    ).ap()
    ext = nc.dram_tensor(
        "ext", [KX, NG, 576], mybir.dt.bfloat16, kind="ExternalInput"
    ).ap()
    outb = nc.dram_tensor(
        "outb", [COUT, NG, NNODES], mybir.dt.bfloat16, kind="ExternalOutput"
    ).ap()

    with tile.TileContext(nc) as tc:
        with (
            tc.tile_pool(name="io", bufs=5) as iop,
            tc.tile_pool(name="ex", bufs=1) as exp_,
            tc.tile_pool(name="ps", bufs=6, space="PSUM") as psp,
            tc.tile_pool(name="ob", bufs=1) as obp,
        ):
            et = exp_.tile([KX, NG, 576], mybir.dt.bfloat16, tag="ext")
            ot4 = obp.tile([COUT, 4, NNODES], mybir.dt.bfloat16, tag="out4")
            ot2 = obp.tile([COUT, 2, NNODES], mybir.dt.bfloat16, tag="out2")
            half = NNODES // 2
            for g in range(NG):
                tail = g >= NG - 2
                t = iop.tile([128, JT, 576], mybir.dt.bfloat16, tag="atr")
                if tail:
                    # split the tail graphs' loads so their last matmuls wait
                    # on a 295KB chunk, not a 590KB one (both halves on SP:
                    # an Act-issued DMA would sit behind out-DMA waits)
                    nc.sync.dma_start(out=t[:, 0:2, :], in_=atr[g, :, 0:2, :])
                    nc.sync.dma_start(out=t[:, 2:4, :], in_=atr[g, :, 2:4, :])
                else:
                    nc.sync.dma_start(out=t[:], in_=atr[g])
                if g == 0:
                    # SWDGE on the otherwise-idle Pool engine; lands between
                    # atr transfers, needed first by graph 0's 5th matmul
                    nc.gpsimd.dma_start(out=et[:], in_=ext[:])
                ps = psp.tile([COUT, NNODES], mybir.dt.float32, tag="ps")
                # tail graphs do the (long-since-loaded) ext matmul FIRST so
                # the final PSUM-group matmul needs only the last chunk
                mm_order = [JT] + list(range(JT)) if tail else list(range(JT)) + [JT]
                for k, jt in enumerate(mm_order):
                    if jt == JT:
                        lhsT, rhs = et[:, g, NNODES:], et[:, g, 0:NNODES]
                    else:
                        lhsT, rhs = t[:, jt, NNODES:], t[:, jt, 0:NNODES]
                    nc.tensor.matmul(
                        ps[:], lhsT=lhsT, rhs=rhs,
                        start=(k == 0), stop=(k == JT),
                    )
                if g < 4:
                    # one 4-graph out DMA: its transfer slots in AFTER the
                    # last atr chunk, keeping the input stream gap-free
                    nc.vector.tensor_copy(ot4[:, g, :], ps[:])
                    if g == 3:
                        nc.scalar.dma_start(out=outb[:, 0:4, :], in_=ot4[:])
                elif g < 6:
                    nc.vector.tensor_copy(ot2[:, g - 4, :], ps[:])
                    if g == 5:
                        nc.scalar.dma_start(out=outb[:, 4:6, :], in_=ot2[:])
                elif g == NG - 2:
                    ots = obp.tile([COUT, NNODES], mybir.dt.bfloat16, tag="o6")
                    nc.vector.tensor_copy(ots[:], ps[:])
                    nc.sync.dma_start(out=outb[:, g, :], in_=ots[:])
                else:
                    # last graph: copy on Act (faster per element than DVE,
                    # and DVE's SEQ is busy dispatching copy6), out on SP
                    o7 = obp.tile([COUT, NNODES], mybir.dt.bfloat16, tag="o7")
                    nc.scalar.copy(o7[:], ps[:])
                    nc.sync.dma_start(out=outb[:, g, :], in_=o7[:])
    nc.compile()
    return nc


def kernel(A, X, A_coeffs, X_coeffs_1, X_coeffs_2):
    global LAST_RESULTS
    from concourse.bass_utils import run_bass_kernel_spmd

    atr, ext = _host_fold(
        np.asarray(A), np.asarray(X), np.asarray(A_coeffs),
        np.asarray(X_coeffs_1), np.asarray(X_coeffs_2),
    )

    if "nc" not in _NC_CACHE:
        _NC_CACHE["nc"] = _build_nc()
    nc = _NC_CACHE["nc"]

    in_maps = [
        {
            "atr": atr[c * NG : (c + 1) * NG],
            "ext": np.ascontiguousarray(ext[:, c * NG : (c + 1) * NG, :]),
        }
        for c in range(NCORES)
    ]
    res = run_bass_kernel_spmd(nc, in_maps, list(range(NCORES)), trace=TRACE)
    LAST_RESULTS = res
    outT = np.stack([r["outb"] for r in res.results])  # [ncores, B, NG, n]
    out = outT.transpose(0, 2, 3, 1).reshape(N, NNODES, COUT)  # [N, n, B]
    return np.ascontiguousarray(out).astype(np.float32)
